# revision 4
# baseline (speedup 1.0000x reference)
"""LlamaMoE (8 experts, top-2) on 8 Trainium2 cores.

Sharding: expert-parallel. Core e holds expert e's full weights and computes
its SwiGLU densely over all T=2048 tokens (output scaled per-token by the
router combine weight, 0 for tokens not routed to e), plus a 1/8
tensor-parallel slice of the always-on base MLP. The router (softmax + top-2
+ renorm) is computed on every core in exact fp32. Per-core partial outputs
are summed and token-sharded with on-device ReduceScatters (one per token
half, so the first overlaps second-half compute); the host only concatenates
slices.

Matmuls run in float32r (1 cycle/row) except the tiny router gate matmul
which uses exact float32 so top-2 selection matches the reference.

Loop structure: tokens split in 2 halves of 1024. Per half: router coefs,
then one sweep over all 25 gate/up weight-pair tiles (22 expert + 3 padded
base-TP), SwiGLU into fp32r `a` tiles, down-projection in ki-groups of <=4
accumulated in PSUM, scaled (expert groups only) and added into a resident
fp32 `osum` [128, 8x1024] accumulator, then DMA to the collective buffer.
"""

import numpy as np

import concourse.bass as bass
import concourse.mybir as mybir
import concourse.tile as tile
from concourse import bacc
from concourse.bass_utils import run_bass_kernel_spmd

N_CORES = 8
H = 1024          # hidden
I = 2816          # expert/base intermediate
E = 8             # experts
T = 2048          # tokens (B*S = 2*1024)
P = 128
KH = H // P       # 8 h-tiles
MI = I // P       # 22 intermediate pair-tiles per expert
ISL = I // N_CORES        # 352 base TP slice
ISLP = 384                # padded to 3*128
KB = ISLP // P            # 3
NM = MI + KB              # 25 pair tiles total per half sweep
TH = T // 2               # 1024 tokens per half
NTT = TH // P             # 8 token sub-tiles per half
NSUB = 2                  # 512-wide matmul sub-chunks per half
SUB = TH // NSUB          # 512
HC = 512                  # output h chunk for down matmul
NHC = H // HC             # 2

F32 = mybir.dt.float32
F32R = mybir.dt.float32r
AF = mybir.ActivationFunctionType
OP = mybir.AluOpType

# ki groups for down-projection accumulation: expert tiles then base tiles
GROUPS = [list(range(0, 4)), list(range(4, 8)), list(range(8, 12)),
          list(range(12, 16)), list(range(16, 20)), list(range(20, 22)),
          list(range(22, 25))]  # last group = base (unscaled)


def _build():
    nc = bacc.Bacc("TRN2", target_bir_lowering=False)

    xh_pk = nc.dram_tensor("xh_pk", [2, P, KH * TH], F32R, kind="ExternalInput")
    gw_pk = nc.dram_tensor("gw_pk", [P, KH * E], F32, kind="ExternalInput")
    onehot = nc.dram_tensor("onehot", [P, E], F32, kind="ExternalInput")
    wgu_pk = nc.dram_tensor("wgu_pk", [MI, P, 2 * KH * P], F32R, kind="ExternalInput")
    wd_pk = nc.dram_tensor("wd_pk", [P, MI * H], F32R, kind="ExternalInput")
    bgu_pk = nc.dram_tensor("bgu_pk", [KB, P, 2 * KH * P], F32R, kind="ExternalInput")
    bwd_pk = nc.dram_tensor("bwd_pk", [P, KB * H], F32R, kind="ExternalInput")
    out_sl = nc.dram_tensor("out_sl", [T // N_CORES, H], F32, kind="ExternalOutput")

    with tile.TileContext(nc) as tc:
        with (
            tc.tile_pool(name="const", bufs=1) as cpool,
            tc.tile_pool(name="xp", bufs=1) as xpool,
            tc.tile_pool(name="os", bufs=1) as ospool,
            tc.tile_pool(name="wg", bufs=3) as wgpool,
            tc.tile_pool(name="wdp", bufs=6) as wdpool,
            tc.tile_pool(name="ap", bufs=8) as apool,
            tc.tile_pool(name="rt", bufs=2) as rtpool,
            tc.tile_pool(name="sgp", bufs=3) as sgpool,
            tc.tile_pool(name="ob", bufs=4) as opool,
            tc.tile_pool(name="ps_gu", bufs=2, space="PSUM") as ps_gu,
            tc.tile_pool(name="ps_dn", bufs=2, space="PSUM") as ps_dn,
            tc.tile_pool(name="ps_rt", bufs=2, space="PSUM") as ps_rt,
            tc.tile_pool(name="dram", bufs=1, space="DRAM") as dpool,
        ):
            # resident constants
            gw_sb = cpool.tile([P, KH * E], F32, tag="gw")
            nc.sync.dma_start(gw_sb[:], gw_pk[:])
            oh_sb = cpool.tile([P, E], F32, tag="oh")
            nc.sync.dma_start(oh_sb[:], onehot[:])
            bwd_sb = cpool.tile([P, KB * H], F32R, tag="bwd")
            nc.sync.dma_start(bwd_sb[:], bwd_pk[:])
            coef = cpool.tile([P, 2 * NTT], F32, tag="coef")

            cc_in = [dpool.tile([TH, H], F32, tag=f"ccin{h}", name=f"ccin{h}") for h in range(2)]
            cc_out = [dpool.tile([TH // N_CORES, H], F32, tag=f"ccout{h}", name=f"ccout{h}") for h in range(2)]

            for half in range(2):
                xh = xpool.tile([P, KH * TH], F32R, tag="xh")
                nc.sync.dma_start(xh[:], xh_pk[half])

                # ---- router: coef[t] for this core's expert, per token sub-tile
                for tt in range(NTT):
                    lg_ps = ps_rt.tile([P, E], F32, tag="lg")
                    for k in range(KH):
                        nc.tensor.matmul(
                            out=lg_ps[:],
                            lhsT=xh[:, k * TH + tt * P : k * TH + (tt + 1) * P].bitcast(F32),
                            rhs=gw_sb[:, k * E : (k + 1) * E],
                            start=(k == 0),
                            stop=(k == KH - 1),
                        )
                    lg = rtpool.tile([P, E], F32, tag="lg_sb")
                    nc.vector.tensor_copy(lg[:], lg_ps[:])
                    mx = rtpool.tile([P, E], F32, tag="mx")
                    nc.vector.max(out=mx[:], in_=lg[:])
                    sc = rtpool.tile([P, 8], F32, tag="sc")
                    m1 = mx[:, 0:1]
                    m2 = mx[:, 1:2]
                    # l_sel = <logits, onehot>
                    nc.vector.tensor_tensor(out=sc[:, 0:E], in0=lg[:], in1=oh_sb[:], op=OP.mult)
                    lsel = rtpool.tile([P, 1], F32, tag="lsel")
                    nc.vector.reduce_sum(out=lsel[:], in_=sc[:, 0:E], axis=mybir.AxisListType.X)
                    # w1 = sigmoid(m1-m2), w2 = 1-w1
                    nc.vector.tensor_sub(out=sc[:, 0:1], in0=m1, in1=m2)
                    nc.scalar.activation(out=sc[:, 1:2], in_=sc[:, 0:1], func=AF.Sigmoid)
                    nc.vector.tensor_scalar(
                        out=sc[:, 2:3], in0=sc[:, 1:2], scalar1=-1.0, scalar2=1.0,
                        op0=OP.mult, op1=OP.add,
                    )
                    # coef = (lsel==m1)*w1 + (lsel==m2)*w2
                    nc.vector.tensor_tensor(out=sc[:, 3:4], in0=lsel[:], in1=m1, op=OP.is_equal)
                    nc.vector.tensor_tensor(out=sc[:, 4:5], in0=lsel[:], in1=m2, op=OP.is_equal)
                    nc.vector.tensor_tensor(out=sc[:, 5:6], in0=sc[:, 3:4], in1=sc[:, 1:2], op=OP.mult)
                    nc.vector.tensor_tensor(out=sc[:, 6:7], in0=sc[:, 4:5], in1=sc[:, 2:3], op=OP.mult)
                    nc.vector.tensor_add(
                        out=coef[:, half * NTT + tt : half * NTT + tt + 1],
                        in0=sc[:, 5:6], in1=sc[:, 6:7],
                    )

                osum = ospool.tile([P, NTT * H], F32, tag="osum")

                for gi, grp in enumerate(GROUPS):
                    is_base = grp[0] >= MI
                    # ---- gate/up + SwiGLU for this group's pair tiles
                    a_tiles = {}
                    for m in grp:
                        if m < MI:
                            wg = wgpool.tile([P, 2 * KH * P], F32R, tag="wg")
                            nc.sync.dma_start(wg[:], wgu_pk[m])
                        else:
                            wg = wgpool.tile([P, 2 * KH * P], F32R, tag="wg")
                            nc.sync.dma_start(wg[:], bgu_pk[m - MI])
                        a_m = apool.tile([P, TH], F32R, tag="a")
                        a_tiles[m] = a_m
                        for s in range(NSUB):
                            g_ps = ps_gu.tile([P, SUB], F32, tag="g")
                            u_ps = ps_gu.tile([P, SUB], F32, tag="u")
                            for k in range(KH):
                                nc.tensor.matmul(
                                    out=g_ps[:],
                                    lhsT=wg[:, k * P : (k + 1) * P],
                                    rhs=xh[:, k * TH + s * SUB : k * TH + (s + 1) * SUB],
                                    start=(k == 0),
                                    stop=(k == KH - 1),
                                )
                            for k in range(KH):
                                nc.tensor.matmul(
                                    out=u_ps[:],
                                    lhsT=wg[:, (KH + k) * P : (KH + k + 1) * P],
                                    rhs=xh[:, k * TH + s * SUB : k * TH + (s + 1) * SUB],
                                    start=(k == 0),
                                    stop=(k == KH - 1),
                                )
                            sg = sgpool.tile([P, SUB], F32, tag="sg")
                            nc.scalar.activation(out=sg[:], in_=g_ps[:], func=AF.Silu)
                            nc.vector.tensor_tensor(
                                out=a_m[:, s * SUB : (s + 1) * SUB],
                                in0=sg[:], in1=u_ps[:], op=OP.mult,
                            )

                    # ---- down projection for this group, accumulate into osum
                    wd_of = {}
                    for ki in grp:
                        if ki < MI:
                            wdk = wdpool.tile([P, H], F32R, tag="wd")
                            nc.sync.dma_start(wdk[:], wd_pk[:, ki * H : (ki + 1) * H])
                            wd_of[ki] = wdk
                    for tt in range(NTT):
                        for hc in range(NHC):
                            e_ps = ps_dn.tile([P, HC], F32, tag="e")
                            for j, ki in enumerate(grp):
                                if ki < MI:
                                    rhs = wd_of[ki][:, hc * HC : (hc + 1) * HC]
                                else:
                                    kb = ki - MI
                                    rhs = bwd_sb[:, kb * H + hc * HC : kb * H + (hc + 1) * HC]
                                nc.tensor.matmul(
                                    out=e_ps[:],
                                    lhsT=a_tiles[ki][:, tt * P : (tt + 1) * P],
                                    rhs=rhs,
                                    start=(j == 0),
                                    stop=(j == len(grp) - 1),
                                )
                            osl = osum[:, tt * H + hc * HC : tt * H + (hc + 1) * HC]
                            if is_base:
                                nc.vector.tensor_add(out=osl, in0=osl, in1=e_ps[:])
                            else:
                                ob = opool.tile([P, HC], F32, tag="ob")
                                nc.vector.tensor_scalar(
                                    out=ob[:], in0=e_ps[:],
                                    scalar1=coef[:, half * NTT + tt : half * NTT + tt + 1],
                                    scalar2=None, op0=OP.mult,
                                )
                                if gi == 0:
                                    nc.vector.tensor_copy(osl, ob[:])
                                else:
                                    nc.vector.tensor_add(out=osl, in0=osl, in1=ob[:])

                # ---- store half partial + reduce-scatter
                for tt in range(NTT):
                    nc.sync.dma_start(
                        cc_in[half][tt * P : (tt + 1) * P, :],
                        osum[:, tt * H : (tt + 1) * H],
                    )
                nc.gpsimd.collective_compute(
                    "ReduceScatter",
                    OP.add,
                    replica_groups=[list(range(N_CORES))],
                    ins=[cc_in[half][:].opt()],
                    outs=[cc_out[half][:].opt()],
                )
                nc.sync.dma_start(
                    out_sl[half * (TH // N_CORES) : (half + 1) * (TH // N_CORES), :],
                    cc_out[half][:],
                )

    nc.compile()
    return nc


_CACHE = {}


def _pack_inputs(x, gate_w, base_wgu, base_wd, exp_wgu, exp_wd):
    xt = np.ascontiguousarray(np.asarray(x, np.float32).reshape(T, H))
    # [half, p, k*c] where xh[h, p, k*TH + c] = x[h*TH + c, k*P + p]
    xh_pk = np.ascontiguousarray(
        xt.reshape(2, TH, KH, P).transpose(0, 3, 2, 1).reshape(2, P, KH * TH)
    )
    gw_pk = np.ascontiguousarray(
        np.asarray(gate_w, np.float32).reshape(KH, P, E).transpose(1, 0, 2).reshape(P, KH * E)
    )

    def pack_gu(w):  # w [H, 2*mi*P] gate|up halves -> [mi, P, 2*KH*P]
        half = w.shape[1] // 2
        mi = half // P
        g = w[:, :half].reshape(KH, P, mi, P)
        u = w[:, half:].reshape(KH, P, mi, P)
        pk = np.stack([g, u], axis=0)  # [gu, k, p, m, c]
        return np.ascontiguousarray(pk.transpose(3, 2, 0, 1, 4).reshape(mi, P, 2 * KH * P))

    def pack_wd(w):  # w [ki*P, H] -> [P, ki*H] inner [hc, c]
        ki = w.shape[0] // P
        return np.ascontiguousarray(
            w.reshape(ki, P, NHC, HC).transpose(1, 0, 2, 3).reshape(P, ki * H)
        )

    per_core = []
    for e in range(N_CORES):
        sl = slice(e * ISL, (e + 1) * ISL)
        bgu = np.zeros((H, 2 * ISLP), np.float32)
        bgu[:, :ISL] = np.asarray(base_wgu, np.float32)[:, :I][:, sl]
        bgu[:, ISLP : ISLP + ISL] = np.asarray(base_wgu, np.float32)[:, I:][:, sl]
        bwd = np.zeros((ISLP, H), np.float32)
        bwd[:ISL] = np.asarray(base_wd, np.float32)[sl, :]
        onehot = np.zeros((P, E), np.float32)
        onehot[:, e] = 1.0
        per_core.append(
            {
                "xh_pk": xh_pk,
                "gw_pk": gw_pk,
                "onehot": onehot,
                "wgu_pk": pack_gu(np.asarray(exp_wgu[e], np.float32)),
                "wd_pk": pack_wd(np.asarray(exp_wd[e], np.float32)),
                "bgu_pk": pack_gu(bgu),
                "bwd_pk": pack_wd(bwd),
            }
        )
    return per_core


def _get_nc():
    if "nc" not in _CACHE:
        _CACHE["nc"] = _build()
    return _CACHE["nc"]


def _unshard(results, shape, dtype):
    y = np.empty((T, H), np.float32)
    q = TH // N_CORES  # 128
    for c in range(N_CORES):
        o = results[c]["out_sl"]
        y[c * q : (c + 1) * q] = o[:q]
        y[TH + c * q : TH + (c + 1) * q] = o[q:]
    return y.reshape(shape).astype(dtype)


def kernel(x, gate_w, base_wgu, base_wd, exp_wgu, exp_wd):
    nc = _get_nc()
    in_maps = _pack_inputs(x, gate_w, base_wgu, base_wd, exp_wgu, exp_wd)
    res = run_bass_kernel_spmd(nc, in_maps, core_ids=list(range(N_CORES)))
    return _unshard(res.results, x.shape, x.dtype)


# revision 6
# speedup vs baseline: 109.0075x; 109.0075x over previous
"""LlamaMoE (8 experts, top-2) on 8 Trainium2 cores.

Sharding: expert-parallel. Core e holds expert e's full weights and computes
its SwiGLU densely over all T=2048 tokens (output scaled per-token by the
router combine weight, 0 for tokens not routed to e), plus a 1/8
tensor-parallel slice of the always-on base MLP. The router (softmax + top-2
+ renorm) is computed on every core in exact fp32. Per-core partial outputs
are summed and token-sharded with on-device ReduceScatters (one per token
half, so the first overlaps second-half compute); the host only concatenates
slices.

Matmuls run in float32r (1 cycle/row) except the tiny router gate matmul
which uses exact float32 so top-2 selection matches the reference.

Loop structure: tokens split in 2 halves of 1024. Per half: router coefs,
then one sweep over all 25 gate/up weight-pair tiles (22 expert + 3 padded
base-TP), SwiGLU into fp32r `a` tiles, down-projection in ki-groups of <=4
accumulated in PSUM, scaled (expert groups only) and added into a resident
fp32 `osum` [128, 8x1024] accumulator, then DMA to the collective buffer.
"""

import numpy as np

import concourse.bass as bass
import concourse.mybir as mybir
import concourse.tile as tile
from concourse import bacc
from concourse.bass_utils import run_bass_kernel_spmd

N_CORES = 8
H = 1024          # hidden
I = 2816          # expert/base intermediate
E = 8             # experts
T = 2048          # tokens (B*S = 2*1024)
P = 128
KH = H // P       # 8 h-tiles
MI = I // P       # 22 intermediate pair-tiles per expert
ISL = I // N_CORES        # 352 base TP slice
ISLP = 384                # padded to 3*128
KB = ISLP // P            # 3
NM = MI + KB              # 25 pair tiles total per half sweep
TH = T // 2               # 1024 tokens per half
NTT = TH // P             # 8 token sub-tiles per half
NSUB = 2                  # 512-wide matmul sub-chunks per half
SUB = TH // NSUB          # 512
HC = 512                  # output h chunk for down matmul
NHC = H // HC             # 2

F32 = mybir.dt.float32
F32R = mybir.dt.float32r
AF = mybir.ActivationFunctionType
OP = mybir.AluOpType

# ki groups for down-projection accumulation: expert tiles then base tiles
GROUPS = [list(range(0, 4)), list(range(4, 8)), list(range(8, 12)),
          list(range(12, 16)), list(range(16, 20)), list(range(20, 22)),
          list(range(22, 25))]  # last group = base (unscaled)


def _build(reps=1):
    nc = bacc.Bacc("TRN2", target_bir_lowering=False)

    xh_pk = nc.dram_tensor("xh_pk", [2, P, KH * TH], F32R, kind="ExternalInput")
    gw_pk = nc.dram_tensor("gw_pk", [P, KH * E], F32, kind="ExternalInput")
    onehot = nc.dram_tensor("onehot", [P, E], F32, kind="ExternalInput")
    wgu_pk = nc.dram_tensor("wgu_pk", [MI, P, 2 * KH * P], F32R, kind="ExternalInput")
    wd_pk = nc.dram_tensor("wd_pk", [P, MI * H], F32R, kind="ExternalInput")
    bgu_pk = nc.dram_tensor("bgu_pk", [KB, P, 2 * KH * P], F32R, kind="ExternalInput")
    bwd_pk = nc.dram_tensor("bwd_pk", [P, KB * H], F32R, kind="ExternalInput")
    out_sl = nc.dram_tensor("out_sl", [T // N_CORES, H], F32, kind="ExternalOutput")

    with tile.TileContext(nc) as tc:
        with (
            tc.tile_pool(name="const", bufs=1) as cpool,
            tc.tile_pool(name="xp", bufs=1) as xpool,
            tc.tile_pool(name="os", bufs=1) as ospool,
            tc.tile_pool(name="wg", bufs=3) as wgpool,
            tc.tile_pool(name="wdp", bufs=6) as wdpool,
            tc.tile_pool(name="ap", bufs=8) as apool,
            tc.tile_pool(name="rt", bufs=2) as rtpool,
            tc.tile_pool(name="sgp", bufs=3) as sgpool,
            tc.tile_pool(name="ob", bufs=4) as opool,
            tc.tile_pool(name="ps_gu", bufs=2, space="PSUM") as ps_gu,
            tc.tile_pool(name="ps_dn", bufs=2, space="PSUM") as ps_dn,
            tc.tile_pool(name="ps_rt", bufs=2, space="PSUM") as ps_rt,
            tc.tile_pool(name="dram", bufs=1, space="DRAM") as dpool,
        ):
            # resident constants
            gw_sb = cpool.tile([P, KH * E], F32, tag="gw")
            nc.sync.dma_start(gw_sb[:], gw_pk[:])
            oh_sb = cpool.tile([P, E], F32, tag="oh")
            nc.sync.dma_start(oh_sb[:], onehot[:])
            bwd_sb = cpool.tile([P, KB * H], F32R, tag="bwd")
            nc.sync.dma_start(bwd_sb[:], bwd_pk[:])
            coef = cpool.tile([P, 2 * NTT], F32, tag="coef")

            cc_in = [dpool.tile([TH, H], F32, tag=f"ccin{h}", name=f"ccin{h}") for h in range(2)]
            cc_out = [dpool.tile([TH // N_CORES, H], F32, tag=f"ccout{h}", name=f"ccout{h}") for h in range(2)]

            for rep in range(reps):
              for half in range(2):
                xh = xpool.tile([P, KH * TH], F32R, tag="xh")
                nc.sync.dma_start(xh[:], xh_pk[half])

                # ---- router: coef[t] for this core's expert, per token sub-tile
                for tt in range(NTT):
                    lg_ps = ps_rt.tile([P, E], F32, tag="lg")
                    for k in range(KH):
                        nc.tensor.matmul(
                            out=lg_ps[:],
                            lhsT=xh[:, k * TH + tt * P : k * TH + (tt + 1) * P].bitcast(F32),
                            rhs=gw_sb[:, k * E : (k + 1) * E],
                            start=(k == 0),
                            stop=(k == KH - 1),
                        )
                    lg = rtpool.tile([P, E], F32, tag="lg_sb")
                    nc.vector.tensor_copy(lg[:], lg_ps[:])
                    mx = rtpool.tile([P, E], F32, tag="mx")
                    nc.vector.max(out=mx[:], in_=lg[:])
                    sc = rtpool.tile([P, 8], F32, tag="sc")
                    m1 = mx[:, 0:1]
                    m2 = mx[:, 1:2]
                    # l_sel = <logits, onehot>
                    nc.vector.tensor_tensor(out=sc[:, 0:E], in0=lg[:], in1=oh_sb[:], op=OP.mult)
                    lsel = rtpool.tile([P, 1], F32, tag="lsel")
                    nc.vector.reduce_sum(out=lsel[:], in_=sc[:, 0:E], axis=mybir.AxisListType.X)
                    # w1 = sigmoid(m1-m2), w2 = 1-w1
                    nc.vector.tensor_sub(out=sc[:, 0:1], in0=m1, in1=m2)
                    nc.scalar.activation(out=sc[:, 1:2], in_=sc[:, 0:1], func=AF.Sigmoid)
                    nc.vector.tensor_scalar(
                        out=sc[:, 2:3], in0=sc[:, 1:2], scalar1=-1.0, scalar2=1.0,
                        op0=OP.mult, op1=OP.add,
                    )
                    # coef = (lsel==m1)*w1 + (lsel==m2)*w2
                    nc.vector.tensor_tensor(out=sc[:, 3:4], in0=lsel[:], in1=m1, op=OP.is_equal)
                    nc.vector.tensor_tensor(out=sc[:, 4:5], in0=lsel[:], in1=m2, op=OP.is_equal)
                    nc.vector.tensor_tensor(out=sc[:, 5:6], in0=sc[:, 3:4], in1=sc[:, 1:2], op=OP.mult)
                    nc.vector.tensor_tensor(out=sc[:, 6:7], in0=sc[:, 4:5], in1=sc[:, 2:3], op=OP.mult)
                    nc.vector.tensor_add(
                        out=coef[:, half * NTT + tt : half * NTT + tt + 1],
                        in0=sc[:, 5:6], in1=sc[:, 6:7],
                    )

                osum = ospool.tile([P, NTT * H], F32, tag="osum")

                for gi, grp in enumerate(GROUPS):
                    is_base = grp[0] >= MI
                    # ---- gate/up + SwiGLU for this group's pair tiles
                    a_tiles = {}
                    for m in grp:
                        if m < MI:
                            wg = wgpool.tile([P, 2 * KH * P], F32R, tag="wg")
                            nc.sync.dma_start(wg[:], wgu_pk[m])
                        else:
                            wg = wgpool.tile([P, 2 * KH * P], F32R, tag="wg")
                            nc.sync.dma_start(wg[:], bgu_pk[m - MI])
                        a_m = apool.tile([P, TH], F32R, tag="a")
                        a_tiles[m] = a_m
                        for s in range(NSUB):
                            g_ps = ps_gu.tile([P, SUB], F32, tag="g")
                            u_ps = ps_gu.tile([P, SUB], F32, tag="u")
                            for k in range(KH):
                                nc.tensor.matmul(
                                    out=g_ps[:],
                                    lhsT=wg[:, k * P : (k + 1) * P],
                                    rhs=xh[:, k * TH + s * SUB : k * TH + (s + 1) * SUB],
                                    start=(k == 0),
                                    stop=(k == KH - 1),
                                )
                            for k in range(KH):
                                nc.tensor.matmul(
                                    out=u_ps[:],
                                    lhsT=wg[:, (KH + k) * P : (KH + k + 1) * P],
                                    rhs=xh[:, k * TH + s * SUB : k * TH + (s + 1) * SUB],
                                    start=(k == 0),
                                    stop=(k == KH - 1),
                                )
                            sg = sgpool.tile([P, SUB], F32, tag="sg")
                            nc.scalar.activation(out=sg[:], in_=g_ps[:], func=AF.Silu)
                            nc.vector.tensor_tensor(
                                out=a_m[:, s * SUB : (s + 1) * SUB],
                                in0=sg[:], in1=u_ps[:], op=OP.mult,
                            )

                    # ---- down projection for this group, accumulate into osum
                    wd_of = {}
                    for ki in grp:
                        if ki < MI:
                            wdk = wdpool.tile([P, H], F32R, tag="wd")
                            nc.sync.dma_start(wdk[:], wd_pk[:, ki * H : (ki + 1) * H])
                            wd_of[ki] = wdk
                    for tt in range(NTT):
                        for hc in range(NHC):
                            e_ps = ps_dn.tile([P, HC], F32, tag="e")
                            for j, ki in enumerate(grp):
                                if ki < MI:
                                    rhs = wd_of[ki][:, hc * HC : (hc + 1) * HC]
                                else:
                                    kb = ki - MI
                                    rhs = bwd_sb[:, kb * H + hc * HC : kb * H + (hc + 1) * HC]
                                nc.tensor.matmul(
                                    out=e_ps[:],
                                    lhsT=a_tiles[ki][:, tt * P : (tt + 1) * P],
                                    rhs=rhs,
                                    start=(j == 0),
                                    stop=(j == len(grp) - 1),
                                )
                            osl = osum[:, tt * H + hc * HC : tt * H + (hc + 1) * HC]
                            if is_base:
                                nc.vector.tensor_add(out=osl, in0=osl, in1=e_ps[:])
                            else:
                                ob = opool.tile([P, HC], F32, tag="ob")
                                nc.vector.tensor_scalar(
                                    out=ob[:], in0=e_ps[:],
                                    scalar1=coef[:, half * NTT + tt : half * NTT + tt + 1],
                                    scalar2=None, op0=OP.mult,
                                )
                                if gi == 0:
                                    nc.vector.tensor_copy(osl, ob[:])
                                else:
                                    nc.vector.tensor_add(out=osl, in0=osl, in1=ob[:])

                # ---- store half partial + reduce-scatter
                for tt in range(NTT):
                    nc.sync.dma_start(
                        cc_in[half][tt * P : (tt + 1) * P, :],
                        osum[:, tt * H : (tt + 1) * H],
                    )
                nc.gpsimd.collective_compute(
                    "ReduceScatter",
                    OP.add,
                    replica_groups=[list(range(N_CORES))],
                    ins=[cc_in[half][:].opt()],
                    outs=[cc_out[half][:].opt()],
                )
                nc.sync.dma_start(
                    out_sl[half * (TH // N_CORES) : (half + 1) * (TH // N_CORES), :],
                    cc_out[half][:],
                )

    nc.compile()
    return nc


_CACHE = {}


def _pack_inputs(x, gate_w, base_wgu, base_wd, exp_wgu, exp_wd):
    xt = np.ascontiguousarray(np.asarray(x, np.float32).reshape(T, H))
    # [half, p, k*c] where xh[h, p, k*TH + c] = x[h*TH + c, k*P + p]
    xh_pk = np.ascontiguousarray(
        xt.reshape(2, TH, KH, P).transpose(0, 3, 2, 1).reshape(2, P, KH * TH)
    )
    gw_pk = np.ascontiguousarray(
        np.asarray(gate_w, np.float32).reshape(KH, P, E).transpose(1, 0, 2).reshape(P, KH * E)
    )

    def pack_gu(w):  # w [H, 2*mi*P] gate|up halves -> [mi, P, 2*KH*P]
        half = w.shape[1] // 2
        mi = half // P
        g = w[:, :half].reshape(KH, P, mi, P)
        u = w[:, half:].reshape(KH, P, mi, P)
        pk = np.stack([g, u], axis=0)  # [gu, k, p, m, c]
        return np.ascontiguousarray(pk.transpose(3, 2, 0, 1, 4).reshape(mi, P, 2 * KH * P))

    def pack_wd(w):  # w [ki*P, H] -> [P, ki*H] inner [hc, c]
        ki = w.shape[0] // P
        return np.ascontiguousarray(
            w.reshape(ki, P, NHC, HC).transpose(1, 0, 2, 3).reshape(P, ki * H)
        )

    per_core = []
    for e in range(N_CORES):
        sl = slice(e * ISL, (e + 1) * ISL)
        bgu = np.zeros((H, 2 * ISLP), np.float32)
        bgu[:, :ISL] = np.asarray(base_wgu, np.float32)[:, :I][:, sl]
        bgu[:, ISLP : ISLP + ISL] = np.asarray(base_wgu, np.float32)[:, I:][:, sl]
        bwd = np.zeros((ISLP, H), np.float32)
        bwd[:ISL] = np.asarray(base_wd, np.float32)[sl, :]
        onehot = np.zeros((P, E), np.float32)
        onehot[:, e] = 1.0
        per_core.append(
            {
                "xh_pk": xh_pk,
                "gw_pk": gw_pk,
                "onehot": onehot,
                "wgu_pk": pack_gu(np.asarray(exp_wgu[e], np.float32)),
                "wd_pk": pack_wd(np.asarray(exp_wd[e], np.float32)),
                "bgu_pk": pack_gu(bgu),
                "bwd_pk": pack_wd(bwd),
            }
        )
    return per_core


def _get_nc():
    if "nc" not in _CACHE:
        _CACHE["nc"] = _build()
    return _CACHE["nc"]


def _unshard(results, shape, dtype):
    y = np.empty((T, H), np.float32)
    q = TH // N_CORES  # 128
    for c in range(N_CORES):
        o = results[c]["out_sl"]
        y[c * q : (c + 1) * q] = o[:q]
        y[TH + c * q : TH + (c + 1) * q] = o[q:]
    return y.reshape(shape).astype(dtype)


def kernel(x, gate_w, base_wgu, base_wd, exp_wgu, exp_wd):
    nc = _get_nc()
    in_maps = _pack_inputs(x, gate_w, base_wgu, base_wd, exp_wgu, exp_wd)
    res = run_bass_kernel_spmd(nc, in_maps, core_ids=list(range(N_CORES)))
    return _unshard(res.results, x.shape, x.dtype)
